# revision 16
# baseline (speedup 1.0000x reference)
"""Trainium2 Bass kernel for nn_Prealign: shared linear proj + ReLU on doc/query,
cross-attention scores, softmax over query positions, attention-weighted sum of
query embeddings.

Sharding: data-parallel over batch B=32 across 8 cores (4 batches/core).
All matmuls run in float32r (full-rate PE streaming, ~1e-4 rel err).

Self-contained: hardcodes shapes B=32, Ld=1024, Lq=128, E=H=768, 8 cores.
"""

import numpy as np

import concourse.bass as bass
import concourse.mybir as mybir
import concourse.tile as tile
from concourse import bacc, bass_isa
from concourse.bass_utils import run_bass_kernel_spmd

F32 = mybir.dt.float32
F32R = mybir.dt.float32r

N_CORES = 8
B = 32
B_LOC = B // N_CORES  # 4 batches per core
LD = 1024
LQ = 128
E = 768
H = 768
EC = E // 128  # 6 e-chunks
HC = H // 128  # 6 h-chunks
DCH = 512  # doc rows per chunk
NSUB = DCH // 128  # 4
N_CHUNKS = B_LOC * (LD // DCH)  # 8 chunks per core

_prog_cache = {}


def build_program(repeat=1):
    if repeat in _prog_cache:
        return _prog_cache[repeat]

    import contextlib

    nc = bacc.Bacc("TRN2", target_bir_lowering=False, debug=False)

    doc_d = nc.dram_tensor("doc", [B_LOC, LD, E], F32, kind="ExternalInput")
    q_d = nc.dram_tensor("query", [B_LOC, LQ, E], F32, kind="ExternalInput")
    w_d = nc.dram_tensor("W", [H, E], F32, kind="ExternalInput")
    b_d = nc.dram_tensor("b", [H], F32, kind="ExternalInput")
    id_d = nc.dram_tensor("ident", [128, 128], F32, kind="ExternalInput")
    out_d = nc.dram_tensor("out", [B_LOC, LD, E], F32, kind="ExternalOutput")

    with tile.TileContext(nc) as tc:
        loop_cm = tc.For_i(0, repeat, 1) if repeat > 1 else contextlib.nullcontext()
        with loop_cm, (
            tc.tile_pool(name="persist", bufs=1)) as persist, (
            tc.tile_pool(name="ptrans", bufs=2, space="PSUM")) as ptrans, (
            tc.tile_pool(name="pmm1", bufs=2, space="PSUM")) as pmm1, (
            tc.tile_pool(name="ppo", bufs=2, space="PSUM")) as ppo, (
            tc.tile_pool(name="dnp", bufs=2)) as dnp:
            # ---------------- persistent tiles ----------------
            ident = persist.tile([128, 128], F32)
            nc.sync.dma_start(ident[:], id_d[:])
            ones_f = persist.tile([128, 8], F32)
            nc.gpsimd.memset(ones_f[:], 1.0)
            ones_r = persist.tile([128, 8], F32R)
            nc.vector.tensor_copy(ones_r[:], ones_f[:])

            b_sb = persist.tile([128, HC], F32)
            nc.sync.dma_start(b_sb[:], b_d.ap().rearrange("(c p) -> p c", p=128))

            # W^T [e_part, ec, h], query_oT [h_part, hc, (b q)], query [q_part, b, e]
            wt = persist.tile([128, EC, H], F32R)
            qot = persist.tile([128, HC, B_LOC * LQ], F32R)
            q_r = persist.tile([128, B_LOC, E], F32R)

            # prefetch first two doc chunks before the setup DMAs
            dn_tiles = {}

            def load_dn(ci):
                bi = ci // (LD // DCH)
                d0 = (ci % (LD // DCH)) * DCH
                dn = dnp.tile([128, NSUB, E], F32, tag="dn")
                nc.sync.dma_start(
                    dn[:],
                    doc_d[bi, d0 : d0 + DCH].rearrange("(s p) e -> p s e", p=128),
                )
                dn_tiles[ci] = dn

            # ---------------- setup: W^T, query proj ----------------
            with tc.tile_pool(name="setup", bufs=1) as setup:
                wn = setup.tile([128, HC, E], F32)
                wn_re = w_d.ap().rearrange("(hc hp) e -> hp hc e", hp=128)
                for hc in range(HC):
                    nc.sync.dma_start(wn[:, hc, :], wn_re[:, hc, :])
                q_sb = setup.tile([128, B_LOC, E], F32)
                q_re = q_d.ap().rearrange("b q e -> q b e")
                for bi in range(B_LOC):
                    nc.sync.dma_start(q_sb[:, bi, :], q_re[:, bi, :])
                load_dn(0)
                load_dn(1)
                nc.vector.tensor_copy(q_r[:], q_sb[:])

                # transpose W: for each hc, all ec blocks -> wt[:, :, hc*128:]
                for hc in range(HC):
                    tp = ppo.tile([128, EC * 128], F32, tag="po")
                    for ec in range(EC):
                        nc.tensor.transpose(
                            tp[:, ec * 128 : (ec + 1) * 128],
                            wn[:, hc, ec * 128 : (ec + 1) * 128],
                            ident[:],
                        )
                    nc.scalar.copy(
                        wt[:, :, hc * 128 : (hc + 1) * 128],
                        tp[:].rearrange("p (ec q) -> p ec q", ec=EC),
                    )

                # transpose query: for each local batch, all ec blocks
                qtr = setup.tile([128, EC, B_LOC * LQ], F32R)
                for bi in range(B_LOC):
                    tp = ppo.tile([128, EC * 128], F32, tag="po")
                    for ec in range(EC):
                        nc.tensor.transpose(
                            tp[:, ec * 128 : (ec + 1) * 128],
                            q_sb[:, bi, ec * 128 : (ec + 1) * 128],
                            ident[:],
                        )
                    nc.vector.tensor_copy(
                        qtr[:, :, bi * 128 : (bi + 1) * 128],
                        tp[:].rearrange("p (ec q) -> p ec q", ec=EC),
                    )

                # query projection: qot[h, (b q)] = relu(W @ query^T + b)
                for hc in range(HC):
                    qp = pmm1.tile([128, B_LOC * LQ], F32, tag="mm1")
                    for ec in range(EC):
                        nc.tensor.matmul(
                            qp[:],
                            wt[:, ec, hc * 128 : (hc + 1) * 128],
                            qtr[:, ec, :],
                            start=(ec == 0),
                            stop=(ec == EC - 1),
                        )
                    nc.scalar.activation(
                        qot[:, hc, :],
                        qp[:],
                        mybir.ActivationFunctionType.Relu,
                        bias=b_sb[:, hc : hc + 1],
                    )

            # ---------------- main loop over doc chunks ----------------
            with (
                tc.tile_pool(name="dtp", bufs=2) as dtp,
                tc.tile_pool(name="dop", bufs=2) as dop,
                tc.tile_pool(name="smp", bufs=2) as smp,
                tc.tile_pool(name="outp", bufs=2) as outp,
            ):
                def emit_mm3(expt, bi, d0):
                    """Second bmm + row-sum + normalize + store, one chunk
                    deferred so PE never waits on the softmax chain."""
                    for s in range(NSUB):
                        po = ppo.tile([128, 1024], F32, tag="po")
                        lhst = expt[:, s * 128 : (s + 1) * 128]
                        nc.tensor.matmul(
                            po[:, 0:512], lhst, q_r[:, bi, 0:512],
                            start=True, stop=True,
                        )
                        nc.tensor.matmul(
                            po[:, 512:768], lhst, q_r[:, bi, 512:768],
                            start=True, stop=True,
                        )
                        nc.tensor.matmul(
                            po[:, 768:776], lhst, ones_r[:],
                            start=True, stop=True,
                        )
                        rs = smp.tile([128, 1], F32, tag="rs")
                        nc.vector.tensor_copy(rs[:], po[:, 768:769])
                        rc = smp.tile([128, 1], F32, tag="rc")
                        nc.vector.reciprocal(rc[:], rs[:])
                        osb = outp.tile([128, E], F32, tag="osb")
                        if s % 2 == 0:
                            nc.scalar.mul(osb[:], po[:, 0:768], rc[:])
                        else:
                            nc.vector.tensor_scalar_mul(osb[:], po[:, 0:768], rc[:])
                        nc.sync.dma_start(
                            out_d[bi, d0 + s * 128 : d0 + (s + 1) * 128],
                            osb[:],
                        )

                pending = None
                for ci in range(N_CHUNKS):
                    bi = ci // (LD // DCH)
                    d0 = (ci % (LD // DCH)) * DCH

                    if ci + 2 < N_CHUNKS:
                        load_dn(ci + 2)
                    dn = dn_tiles.pop(ci)

                    # transpose doc chunk: dtr[e_part, ec, d]
                    dtr = dtp.tile([128, EC, DCH], F32R, tag="dt")
                    for ec in range(EC):
                        tp = ptrans.tile([128, DCH], F32, tag="tp")
                        for s in range(NSUB):
                            nc.tensor.transpose(
                                tp[:, s * 128 : (s + 1) * 128],
                                dn[:, s, ec * 128 : (ec + 1) * 128],
                                ident[:],
                            )
                        nc.vector.tensor_copy(dtr[:, ec, :], tp[:])

                    # MM1: doc proj, relu+bias -> dot[h_part, hc, d]
                    dot = dop.tile([128, HC, DCH], F32R, tag="do")
                    for hc in range(HC):
                        mp = pmm1.tile([128, DCH], F32, tag="mm1")
                        for ec in range(EC):
                            nc.tensor.matmul(
                                mp[:],
                                wt[:, ec, hc * 128 : (hc + 1) * 128],
                                dtr[:, ec, :],
                                start=(ec == 0),
                                stop=(ec == EC - 1),
                            )
                        nc.scalar.activation(
                            dot[:, hc, :],
                            mp[:],
                            mybir.ActivationFunctionType.Relu,
                            bias=b_sb[:, hc : hc + 1],
                        )

                    # MM2: scores^T [q, d] = query_o doc_o^T
                    # (mm1 tag: decouples next chunk's transposes from softmax)
                    sc = pmm1.tile([128, DCH], F32, tag="mm1")
                    for hc in range(HC):
                        nc.tensor.matmul(
                            sc[:],
                            qot[:, hc, bi * 128 : (bi + 1) * 128],
                            dot[:, hc, :],
                            start=(hc == 0),
                            stop=(hc == HC - 1),
                        )

                    # softmax over q (partition dim); MM3 is deferred a full
                    # chunk, so chain latency is hidden
                    sc_sb = smp.tile([128, DCH], F32, tag="scsb")
                    mx = smp.tile([128, DCH], F32, tag="mx")
                    exin = smp.tile([128, DCH], F32, tag="exin")
                    expt = smp.tile([128, DCH], F32R, tag="expt")
                    nc.vector.tensor_copy(sc_sb[:], sc[:])
                    nc.gpsimd.partition_all_reduce(
                        mx[:], sc_sb[:], 128, bass_isa.ReduceOp.max
                    )
                    nc.vector.tensor_sub(exin[:], sc_sb[:], mx[:])
                    nc.scalar.activation(
                        expt[:], exin[:], mybir.ActivationFunctionType.Exp
                    )

                    # previous chunk's MM3 lands here: a full chunk of PE
                    # work separates it from its exp dependencies
                    if pending is not None:
                        emit_mm3(*pending)
                    pending = (expt, bi, d0)

                if pending is not None:
                    emit_mm3(*pending)

    nc.compile()
    _prog_cache[repeat] = nc
    return nc


def run(doc_embed, query_embed, W, b, **kwargs):
    nc = build_program()
    ident = np.eye(128, dtype=np.float32)
    in_maps = []
    for c in range(N_CORES):
        sl = slice(c * B_LOC, (c + 1) * B_LOC)
        in_maps.append(
            {
                "doc": np.ascontiguousarray(doc_embed[sl], dtype=np.float32),
                "query": np.ascontiguousarray(query_embed[sl], dtype=np.float32),
                "W": np.ascontiguousarray(W, dtype=np.float32),
                "b": np.ascontiguousarray(b, dtype=np.float32),
                "ident": ident,
            }
        )
    return run_bass_kernel_spmd(
        nc, in_maps, core_ids=list(range(N_CORES)), **kwargs
    )


def kernel(doc_embed, query_embed, W, b, **_unused):
    res = run(doc_embed, query_embed, W, b)
    return np.concatenate([r["out"] for r in res.results], axis=0)
